# revision 2
# baseline (speedup 1.0000x reference)
"""Causal GQA attention (S=2048, B=2, HQ=32, HKV=8, D=128) on 8 trn2 cores.

Sharding: the 16 (batch, kv-head) pairs are split 2 per core; each pair
carries group=4 query heads -> 8 attention heads per core.

Per head the kernel computes S^T = (Q K^T)^T per 128-wide k-tile into PSUM
(f32r matmuls, 512-col q-chunks), exponentiates on ACT directly into fp8e4
P^T tiles (bias -2 keeps exp under the fp8e4 max; the shift cancels in the
final normalization), applies the causal mask with one gpsimd affine_select
per diagonal slab (sign-safe in fp8), then runs the P*V accumulation and the softmax row-sums
as fp8 DoubleRow matmuls (two 128-row k-slabs contracted per instruction at
half-cycle-per-column rate). Unnormalized out^T and the row sums are DMAd
to DRAM; the host divides and transposes back.

The per-head q range is processed in two 1024-col halves so PSUM fits:
2 staging buffers (2 banks each) + 3 out accumulators + 1 sum bank = 8.
"""

import numpy as np
import ml_dtypes

import concourse.bass as bass
import concourse.mybir as mybir
import concourse.tile as tile
from concourse import bacc, bass_utils

S, B, HQ, HKV, D = 2048, 2, 32, 8, 128
G = HQ // HKV                      # 4 query heads per kv head
NCORES = 8
NPAIRS = B * HKV                   # 16 (batch, kv-head) pairs
BH_PER_CORE = NPAIRS // NCORES     # 2
HEADS_PER_CORE = BH_PER_CORE * G   # 8
SCALE = 1.0 / float(np.sqrt(D))
EXP_BIAS = -2.0                    # exp(s*SCALE - 2): keeps P < 60 (fp8e4 max 240)
QC = 512                           # q-chunk width (PSUM bank)
NQC = S // QC                      # 4
NT = 8                             # 256-wide k-tile pairs per sequence

F32 = mybir.dt.float32
F32R = mybir.dt.float32r
F8 = mybir.dt.float8e4
BF16 = mybir.dt.bfloat16
NP_F8 = ml_dtypes.float8_e4m3
DR = mybir.MatmulPerfMode.DoubleRow


def make_steps():
    """Flat per-head step list: (t, c, chunks-first-flag metadata).

    Per half h (chunks 2h, 2h+1): k-tile pairs t = 0..4h+3, each covering
    chunks max(2h, t//2) .. 2h+1. One step per (t, c).
    """
    steps = []
    for h in range(2):
        for c in (2 * h, 2 * h + 1):
            for t in range(2 * c + 2):
                steps.append((h, t, c))
    return steps


STEPS = make_steps()  # 20 per head


def emit_core_program(tc, qt, kt, v, vb, ones8, sums, ot):
    from contextlib import ExitStack

    nc = tc.nc
    with ExitStack() as ctx:
        _emit(ctx, tc, nc, qt, kt, v, vb, ones8, sums, ot)


def _emit(ctx, tc, nc, qt, kt, v, vb, ones8, sums, ot):
    singles = ctx.enter_context(tc.tile_pool(name="singles", bufs=1))
    kv_pool = ctx.enter_context(tc.tile_pool(name="kv", bufs=2))
    q_pool = ctx.enter_context(tc.tile_pool(name="q", bufs=2))
    pt_pool = ctx.enter_context(tc.tile_pool(name="pt", bufs=3))
    pb_pool = ctx.enter_context(tc.tile_pool(name="pb", bufs=2))
    ob_pool = ctx.enter_context(tc.tile_pool(name="ob", bufs=3))
    sm_pool = ctx.enter_context(tc.tile_pool(name="sm", bufs=3))
    ps_s = ctx.enter_context(tc.tile_pool(name="ps_s", bufs=2, space="PSUM"))
    ps_o = ctx.enter_context(tc.tile_pool(name="ps_o", bufs=3, space="PSUM"))
    ps_sum = ctx.enter_context(tc.tile_pool(name="ps_sum", bufs=1, space="PSUM"))

    ones_sb = singles.tile([128, 2, 32], F8)
    nc.sync.dma_start(out=ones_sb[:], in_=ones8)
    bias_sb = singles.tile([128, 1], F32)
    nc.vector.memset(bias_sb[:], EXP_BIAS)
    onesb = singles.tile([128, 1], BF16)
    nc.vector.memset(onesb[:], 1.0)

    kt_sb = []
    v_sb = []
    vb_sb = []
    for bh in range(BH_PER_CORE):
        ktile = kv_pool.tile([D, S], F32R, tag="kt", name=f"kt_{bh}")
        nc.sync.dma_start(out=ktile[:], in_=kt[bh])
        vtile = kv_pool.tile([128, NT, 2, D], F8, tag="v", name=f"v_{bh}")
        nc.sync.dma_start(out=vtile[:], in_=v[bh])
        vbtile = kv_pool.tile([128, 2, D], BF16, tag="vb", name=f"vb_{bh}")
        nc.sync.dma_start(out=vbtile[:], in_=vb[bh])
        kt_sb.append(ktile)
        v_sb.append(vtile)
        vb_sb.append(vbtile)

    q_tiles = {}

    def fetch_q(head):
        if head < HEADS_PER_CORE and head not in q_tiles:
            qtile = q_pool.tile([D, S], F32R, tag="q", name=f"q_{head}")
            nc.sync.dma_start(out=qtile[:], in_=qt[head])
            q_tiles[head] = qtile

    fetch_q(0)

    # Build the global step list across heads.
    work = []
    for head in range(HEADS_PER_CORE):
        for i, (h, t, c) in enumerate(STEPS):
            work.append((head, h, t, c, i == 0))

    state = {}  # per in-flight step index -> dict(s_t=..., p_t=...)

    def emit_qk(idx):
        head, h, t, c, head_start = work[idx]
        bh = head // G
        if head_start:
            fetch_q(head + 1)
        # out accumulators + sum tile, allocated lazily per half / head
        if ("sum", head, c) not in state:
            state[("sum", head, c)] = ps_sum.tile(
                [32, QC], F32, tag="sum", name=f"sum_{head}_{c}")
        for cc in (2 * h, 2 * h + 1):
            if ("o", head, cc) not in state:
                state[("o", head, cc)] = ps_o.tile(
                    [128, QC], F32, tag="o", name=f"o_{head}_{cc}")
        s_t = ps_s.tile([128, 2, QC], F32, tag="s", name=f"s_{head}_{t}_{c}")
        q_sb = q_tiles[head]
        for s in (0, 1):
            klo = 256 * t + 128 * s
            nc.tensor.matmul(
                out=s_t[:, s, :],
                lhsT=kt_sb[bh][:, klo:klo + 128],
                rhs=q_sb[:, QC * c:QC * (c + 1)],
                start=True, stop=True,
            )
        state[idx] = {"s_t": s_t}

    def emit_exp(idx):
        head, h, t, c, _ = work[idx]
        st = state[idx]
        # (t=0, c=0) runs in bf16: the first q rows average only a few
        # softmax terms, so fp8 P/V noise does not cancel there
        bf = t == 0 and c == 0
        if bf:
            p_t = pb_pool.tile([128, 2, QC], BF16, tag="pb", name=f"pb_{head}")
        else:
            p_t = pt_pool.tile([128, 2, QC], F8, tag="p",
                               name=f"p_{head}_{t}_{c}")
        nc.scalar.activation(
            p_t[:], st["s_t"][:],
            mybir.ActivationFunctionType.Exp, scale=SCALE, bias=bias_sb[:])
        if c == t // 2:
            # causal mask on the diagonal slabs: keep q >= k, else 0
            for s in (0, 1):
                klo = 256 * t + 128 * s
                nc.gpsimd.affine_select(
                    out=p_t[:, s, :], in_=p_t[:, s, :],
                    pattern=[[1, QC]], base=QC * c - klo,
                    channel_multiplier=-1,
                    compare_op=mybir.AluOpType.is_ge, fill=0.0)
        st["p_t"] = p_t

    def emit_pv(idx):
        head, h, t, c, _ = work[idx]
        bh = head // G
        st = state.pop(idx)
        p_t = st["p_t"]
        o_t = state[("o", head, c)]
        sum_t = state[("sum", head, c)]
        first = t == 0
        last = t == 2 * c + 1
        # PSUM start=True zeroing is bank-granular: only the first matmul
        # into each bank may set start; later writes into the same bank
        # accumulate over its pending-zero state.
        if t == 0 and c == 0:
            for s in (0, 1):
                nc.tensor.matmul(
                    out=o_t[:], lhsT=vb_sb[bh][:, s, :], rhs=p_t[:, s, :],
                    start=s == 0, stop=False, skip_group_check=True,
                )
                nc.tensor.matmul(
                    out=sum_t[0:1, :], lhsT=onesb[:], rhs=p_t[:, s, :],
                    start=s == 0, stop=False, skip_group_check=True,
                )
        else:
            for hc in (0, 1):
                rhs = p_t[:, :, 256 * hc:256 * (hc + 1)]
                nc.tensor.matmul(
                    out=o_t[:, 256 * hc:256 * (hc + 1)],
                    lhsT=v_sb[bh][:, t], rhs=rhs,
                    start=first and hc == 0, stop=last, perf_mode=DR,
                    skip_group_check=True,
                )
            for hc in (0, 1):
                rhs = p_t[:, :, 256 * hc:256 * (hc + 1)]
                nc.tensor.matmul(
                    out=sum_t[0:32, 256 * hc:256 * (hc + 1)],
                    lhsT=ones_sb[:], rhs=rhs,
                    start=first and hc == 0, stop=last, perf_mode=DR,
                    skip_group_check=True,
                )
        if last:
            osb = ob_pool.tile([128, QC], F32, tag="ob", name=f"ob_{head}_{c}")
            nc.vector.tensor_scalar_add(osb[:], o_t[:], 0.0)
            nc.sync.dma_start(
                out=ot[head][:, QC * c:QC * (c + 1)], in_=osb[:])
            ssb = sm_pool.tile([1, QC], F32, tag="sm", name=f"sm_{head}_{c}")
            nc.vector.tensor_scalar_add(ssb[:], sum_t[0:1, :], 0.0)
            nc.sync.dma_start(out=sums[head, c], in_=ssb[:])
            del state[("o", head, c)]
            del state[("sum", head, c)]

    emit_qk(0)
    for i in range(len(work)):
        emit_exp(i)
        if i + 1 < len(work):
            emit_qk(i + 1)
        emit_pv(i)


_CACHED_NC = None


def build_program():
    global _CACHED_NC
    if _CACHED_NC is not None:
        return _CACHED_NC
    nc = bacc.Bacc("TRN2", target_bir_lowering=False, debug=False,
                   num_devices=NCORES)
    qt = nc.dram_tensor("qt", [HEADS_PER_CORE, D, S], F32R,
                        kind="ExternalInput").ap()
    kt = nc.dram_tensor("kt", [BH_PER_CORE, D, S], F32R,
                        kind="ExternalInput").ap()
    v = nc.dram_tensor("v", [BH_PER_CORE, 128, NT, 2, D], F8,
                       kind="ExternalInput").ap()
    ones8 = nc.dram_tensor("ones8", [128, 2, 32], F8,
                           kind="ExternalInput").ap()
    vb = nc.dram_tensor("vb", [BH_PER_CORE, 128, 2, D], mybir.dt.bfloat16,
                        kind="ExternalInput").ap()
    sums = nc.dram_tensor("sums", [HEADS_PER_CORE, NQC, QC], F32,
                          kind="ExternalOutput").ap()
    ot = nc.dram_tensor("ot", [HEADS_PER_CORE, D, S], F32,
                        kind="ExternalOutput").ap()
    with tile.TileContext(nc) as tc:
        emit_core_program(tc, qt, kt, v, vb, ones8, sums, ot)
    nc.compile()
    _CACHED_NC = nc
    return nc


def shard_inputs(query, key, value):
    """Full inputs -> list of 8 per-core in_maps (host-side relayout only)."""
    query = np.asarray(query, dtype=np.float32)
    key = np.asarray(key, dtype=np.float32)
    value = np.asarray(value, dtype=np.float32)

    # Q: [S,B,HQ,D] -> [B*HKV, G, D, S]
    qtall = np.ascontiguousarray(
        query.reshape(S, B, HKV, G, D).transpose(1, 2, 3, 4, 0)
    ).reshape(NPAIRS, G, D, S)
    # K: [S,B,HKV,D] -> [B*HKV, D, S]
    ktall = np.ascontiguousarray(
        key.transpose(1, 2, 3, 0)).reshape(NPAIRS, D, S)
    # V: [S,B,HKV,D] -> [B*HKV, r=128, t=8, s=2, D] fp8
    #    (k = 256 t + 128 s + r)
    vall_f32 = np.ascontiguousarray(
        value.reshape(NT, 2, 128, B, HKV, D).transpose(3, 4, 2, 0, 1, 5)
    ).reshape(NPAIRS, 128, NT, 2, D)
    vall = vall_f32.astype(NP_F8)
    vall_bf = np.ascontiguousarray(vall_f32[:, :, 0]).astype(ml_dtypes.bfloat16)

    ones8 = np.ones([128, 2, 32], NP_F8)
    vball = vall_bf  # [NPAIRS, 128, 2, D] bf16 (t=0 pair)

    in_maps = []
    for c in range(NCORES):
        p0 = BH_PER_CORE * c
        p1 = p0 + BH_PER_CORE
        in_maps.append({
            "qt": np.ascontiguousarray(qtall[p0:p1].reshape(HEADS_PER_CORE, D, S)),
            "kt": np.ascontiguousarray(ktall[p0:p1]),
            "v": np.ascontiguousarray(vall[p0:p1]),
            "ones8": ones8,
            "vb": np.ascontiguousarray(vball[p0:p1]),
        })
    return in_maps


def unshard_output(results):
    """8 per-core {'ot', 'sums'} -> full [S, B, HQ, D] (normalize on host)."""
    ot = np.stack([r["ot"] for r in results])            # [8, 8, D, S]
    sm = np.stack([r["sums"] for r in results])          # [8, 8, 4, 512]
    ot = ot / sm.reshape(NCORES, HEADS_PER_CORE, 1, S)
    ot = ot.reshape(B, HKV, G, D, S)
    out = np.ascontiguousarray(ot.transpose(4, 0, 1, 2, 3))  # [S,B,HKV,G,D]
    return out.reshape(S, B, HQ, D)


def kernel(query, key, value, _trace=False, _return_bkr=False):
    nc = build_program()
    in_maps = shard_inputs(query, key, value)
    bkr = bass_utils.run_bass_kernel_spmd(
        nc, in_maps, core_ids=list(range(NCORES)), trace=_trace)
    out = unshard_output(bkr.results)
    if _return_bkr:
        return out, bkr
    return out


if __name__ == "__main__":
    q = np.random.randn(S, B, HQ, D).astype(np.float32)
    k = np.random.randn(S, B, HKV, D).astype(np.float32)
    vv = np.random.randn(S, B, HKV, D).astype(np.float32)
    o = kernel(q, k, vv)
    print("out", o.shape, o.dtype, float(np.abs(o).max()))
